# revision 39
# baseline (speedup 1.0000x reference)
"""Multi-head attention forward on 8 Trainium2 NeuronCores.

Problem: nn_Attention_89060441850459
  inputs [8, 1024, 768] f32, w_qkv [768, 2304], w_proj [768, 768], b_proj [768]
  out = proj(softmax(q k^T / sqrt(64)) v) + b_proj,  H=12 heads, hd=64

Sharding: data parallel over batch — each of the 8 cores computes one batch
element end-to-end; weights replicated. No collectives.

Per-core dataflow (fp16 matmul operands, fp32 PSUM accumulation):

  1. xT[d, n]   = PE-transpose of x[n, d]                       (d-major x)
  2. v[n, c]    = x @ w_qkv[:, 1536:]          (s-major, heads padded with a
                  ones-column per head -> [1024, 12*65] so the PV matmul also
                  produces the softmax denominator for free)
  3. qkT[m, n]  = w_qkv[:, :1536].T @ xT      (q/k head-dim-major: [1536, 1024])
  4. per head-PAIR p, per (qpos-half n2, key-chunk m):
       S^T halves of the two heads -> two [128,512] 1-bank PSUM tiles via
         row-tiled matmuls that run concurrently in the PE array
       E = exp(S^T / 8): head a exact exp on ACT, head b f16-Schraudolph on
         the DVE (fused mult+add f32->i16 convert writing f16 exp2 bit
         patterns, +-3% sawtooth; each head's softmax is uniformly exact or
         approximate so the bias normalizes out; end-to-end rel err ~1e-2
         vs the 2e-2 gate)
       O_aug[65, 512] += v_pad_m[:, h].T @ E-half  (PSUM-accumulated over m;
                                                    row 64 = sum_k E = Z)
     then O^T_h = O_aug[0:64] * broadcast(1/Z). Pairs 0-4: reciprocal on a
       [128,8] reshape via DRAM bounces (latency hidden under the window),
       muls on GPSIMD. Last pair: zero-DMA path — 1/Z = exp(-ln Z) on ACT
       (combined exp+ln table set loaded manually at t=0), rank-1 PE matmul
       broadcast, muls on DVE — keeps the PE from idling into a HAM
       re-throttle before the proj tail.
  5. y = O^T-stacked.T @ w_proj + b_proj (PSUM-accumulated tail).

  Scheduling: a PE warmup burst at t=0 (hidden under the input DMA) trips
  the HAM clock gate to 2.4 GHz; x is DMAed on two queues, cast on the DVE,
  and copied out of PSUM one merged 3D copy per chunk. The attention window
  runs a 3-deep software pipeline with THREE 1-bank PSUM slots per S half,
  so the S(t) <- exp(t-3) slot-recycle chain stays off the critical path
  and the window paces at the PE matmul stream.
"""

import sys

if "/opt/trn_rl_repo" not in sys.path:
    sys.path.insert(0, "/opt/trn_rl_repo")

from contextlib import ExitStack

import numpy as np

import concourse.bass as bass
import concourse.mybir as mybir
import concourse.tile as tile
from concourse import bacc
from concourse.masks import make_identity

B, N, D = 8, 1024, 768
H = 12
HD = D // H  # 64
NCORES = 8
P = 128
NT = N // P  # 8 seq chunks
DC = D // P  # 6 d chunks
F32 = mybir.dt.float32
F16 = mybir.dt.float16
I16 = mybir.dt.int16
SCALE = HD**-0.5
# f16 Schraudolph exp2 constants: bits = round(S * A + B), reinterpret i16->f16
EXP_A = float(1024.0 * np.log2(np.e) * SCALE)
EXP_B = float(15 * 1024 - 44.0)
# act_info.json set index of natural_log_exp_and_others (exp AND ln in one set)
ACT_SET_LN_EXP = 6


def build_attention(ctx: ExitStack, tc: "tile.TileContext", x, w_qkv, w_proj, b_proj, y):
    nc = tc.nc
    exp = mybir.ActivationFunctionType.Exp
    ln = mybir.ActivationFunctionType.Ln

    perm = ctx.enter_context(tc.tile_pool(name="perm", bufs=1))
    # PSUM: two 1-bank S-half tags with THREE bufs each (6 banks) + two
    # oaug banks. 3 S slots per half keep the slot-recycle chain
    # (S(t) waits exp(t-3)) two chunks of slack off the PE critical path.
    psum = ctx.enter_context(tc.tile_pool(name="psum", bufs=3, space="PSUM"))
    att_psum = ctx.enter_context(tc.tile_pool(name="attps", bufs=2, space="PSUM"))
    zspill = ctx.enter_context(tc.tile_pool(name="zspill", bufs=2, space="DRAM"))
    tmp = ctx.enter_context(tc.tile_pool(name="tmp", bufs=1))
    att = ctx.enter_context(tc.tile_pool(name="att", bufs=2))

    nc.scalar.add_instruction(
        mybir.InstLoadActFuncSet(
            name=nc.get_next_instruction_name(), ins=[], outs=[],
            act_func_set_id=ACT_SET_LN_EXP,
        )
    )

    def sa_tile(name):
        return psum.tile([P, 512], F32, tag="sa", name=name, bufs=3)

    def sb_tile(name):
        return psum.tile([P, 512], F32, tag="sb", name=name, bufs=3)

    # ---------------- PE warmup (HAM clock gate) ----------------
    # ~15 dummy 512-col matmuls on a zeroed tile keep the PE busy well past
    # the ~3.4us HAM SHORT window while the first x chunks are still in
    # flight, so the lead runs at 2.4 GHz instead of 1.2 (a 9-matmul burst
    # measured as NOT tripping the gate — K=8/8 only arrived at 37us).
    warm16 = perm.tile([P, 512], F16, tag="warm", name="warm16")
    nc.gpsimd.memset(warm16, 0)
    warm_ps = sa_tile("warmps")
    for _ in range(9):
        nc.tensor.matmul(
            warm_ps, lhsT=warm16[:, 0:P], rhs=warm16,
            start=True, stop=True, skip_group_check=True,
        )

    # f32 identity: the x transposes run on the raw f32 input (no pre-cast),
    # the PSUM->SBUF merged copies do the f32->f16 conversion instead
    identity = perm.tile([P, P], F16, tag="identity", name="identity")
    make_identity(nc, identity)
    ones1 = perm.tile([1, HD], F16, tag="ones1", name="ones1")
    nc.vector.memset(ones1, 1.0)

    # persistent SBUF arrays
    qkT = [perm.tile([P, N], F16, tag=f"qkT{m}", name=f"qkT{m}") for m in range(12)]
    vpad = [perm.tile([P, H * (HD + 1)], F16, tag=f"vpad{i}", name=f"vpad{i}") for i in range(NT)]
    oT = [perm.tile([P, N], F16, tag=f"oT{j}", name=f"oT{j}") for j in range(DC)]

    # ---------------- loads, casts, transposes ----------------
    wq = [tmp.tile([P, 3 * D], F16, tag=f"wq{k}", name=f"wq{k}") for k in range(DC)]
    wp = [att.tile([P, D], F16, tag=f"wp{k}", name=f"wp{k}", bufs=1) for k in range(DC)]
    # all six d-chunks of xT in one tile so each x chunk needs ONE psum->sbuf
    # copy (strided 3D dest) instead of six
    xT_all = tmp.tile([P, DC * N], F16, tag="xT", name="xT")
    xT3 = xT_all.rearrange("p (j n) -> p j n", n=N)
    xT = [xT_all[:, j * N : (j + 1) * N] for j in range(DC)]
    xin = ctx.enter_context(tc.tile_pool(name="xin", bufs=3))
    # x first, split across the sync and gpsimd DMA queues. The transposes
    # run on the raw f32 chunks (four d-blocks into a 1-bank sa tile, two
    # into sb); the two merged 3D copies convert f32->f16 on the way out.
    for i in range(NT):
        xt = xin.tile([P, D], F32, tag="x", name="xt", bufs=3)
        eng = nc.sync if i % 2 == 0 else nc.gpsimd
        eng.dma_start(out=xt, in_=x[i * P : (i + 1) * P, :])
        xt16 = xin.tile([P, D], F16, tag="x16", name="xt16", bufs=2)
        nc.vector.tensor_copy(xt16, xt)
        pt = sa_tile("tps")
        pt16 = pt.bitcast(F16)
        for j in range(DC):
            nc.tensor.transpose(
                pt16[:, j * P : (j + 1) * P], xt16[:, j * P : (j + 1) * P], identity
            )
        nc.vector.tensor_copy(
            xT3[:, :, i * P : (i + 1) * P],
            pt16[:, 0 : DC * P].rearrange("p (j c) -> p j c", c=P),
        )

    # weights arrive f32 and DMA cannot cast: stage through f32 tiles and
    # cast on the scalar engine. wv split scalar/sync (it gates the v
    # stream), wqk 3-way — the pre-window DMA (x 3MB + wv 2.3MB + wqk
    # 4.6MB) is the lead's hard floor, so balance the queues at ~3MB each.
    for k in range(DC):
        w32v = xin.tile([P, D], F32, tag="wp32", name="w32v", bufs=2)
        nc.scalar.dma_start(out=w32v, in_=w_qkv[k * P : (k + 1) * P, 2 * D : 3 * D])
        nc.scalar.copy(wq[k][:, 2 * D : 3 * D], w32v)
    for k in range(DC):
        w32qk = xin.tile([P, 2 * D], F32, tag="w32", name="w32qk", bufs=3)
        eng = nc.gpsimd if k % 2 == 0 else nc.scalar
        eng.dma_start(out=w32qk, in_=w_qkv[k * P : (k + 1) * P, 0 : 2 * D])
        nc.scalar.copy(wq[k][:, 0 : 2 * D], w32qk)
    brep = att.tile([P, D], F32, tag="brep", name="brep", bufs=1)
    nc.sync.dma_start(out=brep, in_=b_proj.partition_broadcast(P))

    # bridge burst: the PE sits idle ~20-28us waiting on the wv casts and
    # the HAM clock gate re-throttles to 1.2 GHz right as v starts. These
    # dummy matmuls have no dependencies, so they execute exactly in that
    # hole and keep the array warm (they cost nothing when v is ready).
    bridge_ps = att_psum.tile([P, N // 2], F32, tag="oaug", name="bridge", bufs=2)
    for _ in range(32):
        nc.tensor.matmul(
            bridge_ps, lhsT=warm16[:, 0:P], rhs=warm16,
            start=True, stop=True, skip_group_check=True,
        )

    # ---------------- v then qkT (dense PE lead) ----------------
    # v[i][n, c] = sum_k x[n, k] w_qkv[k, 1536+c], head-padded with a
    # per-head ones column (so the PV matmul also produces the softmax Z)
    for i in range(NT):
        vA = sa_tile("vA")
        vB = sb_tile("vB")
        for k in range(DC):
            for ps_, c0, cw in ((vA, 0, 512), (vB, 512, 256)):
                nc.tensor.matmul(
                    ps_[:, 0:cw],
                    lhsT=xT[k][:, i * P : (i + 1) * P],
                    rhs=wq[k][:, 2 * D + c0 : 2 * D + c0 + cw],
                    start=(k == 0),
                    stop=(k == DC - 1),
                    skip_group_check=True,
                )
        vp3 = vpad[i].rearrange("p (h c) -> p h c", c=HD + 1)
        nc.vector.tensor_copy(
            vp3[:, 0:8, 0:HD], vA.rearrange("p (h c) -> p h c", c=HD)
        )
        nc.vector.tensor_copy(
            vp3[:, 8:12, 0:HD], vB[:, 0:256].rearrange("p (h c) -> p h c", c=HD)
        )
        nc.vector.tensor_scalar(
            vp3[:, :, HD : HD + 1], vp3[:, :, 0:1], 0.0, 1.0,
            mybir.AluOpType.mult, mybir.AluOpType.add,
        )

    # qkT[m][dm, n] = sum_k w_qkv[k, m*128+dm] * xT[k, n]; psum->sbuf copies
    # on the DVE (the scalar queue is busy with the bandwidth-gated wqk
    # casts). Only m=0 and m=6 are built before the attention stream starts;
    # the other ten jobs interleave into the early window, where the PE is
    # the pacer and the exp engines have slack.
    def qkT_job(m):
        qA = sa_tile("qA")
        qB = sb_tile("qB")
        for k in range(DC):
            for ps_, n2 in ((qA, 0), (qB, 1)):
                nc.tensor.matmul(
                    ps_,
                    lhsT=wq[k][:, m * P : (m + 1) * P],
                    rhs=xT[k][:, n2 * 512 : (n2 + 1) * 512],
                    start=(k == 0),
                    stop=(k == DC - 1),
                    skip_group_check=True,
                )
        nc.vector.tensor_copy(qkT[m][:, 0:512], qA)
        nc.vector.tensor_copy(qkT[m][:, 512:1024], qB)

    for m in (0, 6):
        qkT_job(m)

    # wp loads/casts also ride inside the window (needed only at the proj
    # tail); issue and cast are separate hooks so a cast never waits on
    # in-flight DMA data while blocking the scalar queue's exp stream
    wp32s = {}

    def wp_issue(k):
        # gpsimd DMA queue: idle in-window, so the 2.3MB of wp transfers
        # never contend with the norm-chain DMAs on the sync queue
        wp32s[k] = xin.tile([P, D], F32, tag="wp32", name="wp32", bufs=2)
        nc.gpsimd.dma_start(out=wp32s[k], in_=w_proj[k * P : (k + 1) * P, :])

    def wp_cast(k):
        nc.scalar.copy(wp[k], wp32s.pop(k))

    # ---------------- attention ----------------
    # Head PAIRS (heads 2p, 2p+1 share the qkT pair tile: head a on
    # partitions 0:64, head b on 64:128). A chunk is (pair, qpos-half n2,
    # key-block m) — n2 OUTER so only two oaug accumulators live at a time
    # (2 PSUM banks) and boundary frees hide under the stream.
    chunks = [(p, n2, m) for p in range(H // 2) for n2 in range(2) for m in range(NT)]
    T = len(chunks)
    oaug = {}
    sps = {}
    epool = {}

    def emit_s(t):
        p, n2, m = chunks[t]
        spa = sa_tile("spa")
        spb = sb_tile("spb")
        sps[t] = (spa, spb)
        for half, sp in ((0, spa), (1, spb)):
            row = half * HD
            kT_h = qkT[6 + p][row : row + HD, :]
            qT_h = qkT[p][row : row + HD, :]
            nc.tensor.matmul(
                sp,
                lhsT=kT_h[:, m * P : (m + 1) * P],
                rhs=qT_h[:, n2 * 512 : (n2 + 1) * 512],
                start=True,
                stop=True,
            )

    def emit_exp(t):
        # half-split: head a exact exp on ACT, head b f16-Schraudolph on the
        # DVE — halves the exp latency per chunk and keeps each head's
        # softmax uniformly exact or approximate
        spa, spb = sps.pop(t)
        ea = att.tile([P, 512], F16, tag="ea", name="ea", bufs=5)
        eb = att.tile([P, 512], F16, tag="eb", name="eb", bufs=5)
        epool[t] = (ea, eb)
        nc.scalar.activation(ea, spa, exp, scale=SCALE)
        nc.vector.tensor_scalar(
            eb.bitcast(I16), spb, EXP_A, EXP_B,
            mybir.AluOpType.mult, mybir.AluOpType.add,
        )

    def emit_o(t):
        p, n2, m = chunks[t]
        if m == 0:
            # lazy alloc at the PV stage so the wait on the outgoing half's
            # osb copies never gets ahead of the PE stream
            for h in (2 * p, 2 * p + 1):
                oaug[(h, n2)] = att_psum.tile(
                    [HD + 1, N // 2], F32, tag="oaug", name="oaug", bufs=2
                )
        ea, eb = epool.pop(t)
        for half, e in ((0, ea), (1, eb)):
            h = 2 * p + half
            vl = vpad[m][:, h * (HD + 1) : (h + 1) * (HD + 1)]
            nc.tensor.matmul(
                oaug[(h, n2)],
                lhsT=vl,
                rhs=e,
                start=(m == 0),
                stop=(m == NT - 1),
                skip_group_check=True,
            )
        if m == NT - 1:
            emit_osb(2 * p, n2)
            emit_osb(2 * p + 1, n2)
            if n2 == 1:
                if p == H // 2 - 1:
                    emit_norm_fast(2 * p)
                else:
                    emit_norm(2 * p)
                    emit_norm(2 * p + 1)

    osbs = {}

    def emit_osb(h, half2):
        # Copy O-half + its Z row to SBUF (frees one PSUM bank). Head b's
        # copy goes to the scalar engine so both heads copy in parallel.
        oa = oaug.pop((h, half2))
        osb = att.tile([HD + 1, N // 2], F32, tag="osb", name="osb", bufs=8)
        if h % 2 == 0:
            nc.vector.tensor_copy(osb, oa)
        else:
            nc.scalar.copy(osb, oa)
        osbs[(h, half2)] = osb

    def emit_norm(h):
        # pairs 0-4: reciprocal on a [128,8] reshape via DRAM bounces; the
        # chain latency hides under the remaining attention window. The
        # final muls run on GPSIMD (SBUF-only engine, otherwise idle).
        row = (h % 2) * HD
        oA = osbs.pop((h, 0))
        oB = osbs.pop((h, 1))
        zd = zspill.tile([1, N], F32, tag="zd", name="zd", bufs=2)
        nc.sync.dma_start(out=zd[0:1, 0 : N // 2], in_=oA[HD : HD + 1, :])
        nc.sync.dma_start(out=zd[0:1, N // 2 : N], in_=oB[HD : HD + 1, :])
        z8 = att.tile([P, N // P], F32, tag="z8", name="z8")
        nc.sync.dma_start(out=z8, in_=zd.rearrange("o (p f) -> (o p) f", p=P))
        r8 = att.tile([P, N // P], F32, tag="r8", name="r8")
        nc.vector.reciprocal(r8, z8)
        rd = zspill.tile([1, N], F32, tag="rd", name="rd", bufs=2)
        nc.sync.dma_start(out=rd.rearrange("o (p f) -> (o p) f", p=P), in_=r8)
        zrep = att.tile([HD, N], F32, tag="zrep", name="zrep")
        nc.sync.dma_start(out=zrep, in_=rd[0, :].partition_broadcast(HD))
        nc.gpsimd.tensor_tensor(
            oT[h // 2][row : row + HD, 0 : N // 2], oA[0:HD, :], zrep[:, 0 : N // 2],
            mybir.AluOpType.mult,
        )
        nc.gpsimd.tensor_tensor(
            oT[h // 2][row : row + HD, N // 2 : N], oB[0:HD, :], zrep[:, N // 2 : N],
            mybir.AluOpType.mult,
        )

    def emit_norm_fast(h0):
        # last pair: zero-DMA normalization so the PE flows straight into the
        # proj tail. 1/Z = exp(-ln Z) on ACT (one combined table set), rank-1
        # PE matmuls broadcast it to [64,512], muls on the DVE.
        j = h0 // 2
        quads = [(h0, 0), (h0, 1), (h0 + 1, 0), (h0 + 1, 1)]
        # engine partition offsets must be 32-aligned: space the 4 Z rows at
        # partitions 0/32/64/96
        z4 = att.tile([97, N // 2], F32, tag="z4", name="z4")
        for i, (h, half2) in enumerate(quads):
            nc.vector.tensor_copy(
                z4[32 * i : 32 * i + 1, :], osbs[(h, half2)][HD : HD + 1, :]
            )
        l4 = att.tile([97, N // 2], F32, tag="l4", name="l4")
        nc.scalar.activation(l4, z4, ln)
        r4 = att.tile([97, N // 2], F16, tag="r4", name="r4")
        nc.scalar.activation(r4, l4, exp, scale=-1.0)
        zps = {}
        for i, (h, half2) in enumerate(quads):
            zps[(h, half2)] = sa_tile("zps") if half2 == 0 else sb_tile("zps")
            rr = att.tile([1, N // 2], F16, tag="rrow", name="rrow", bufs=4)
            nc.vector.tensor_copy(rr, r4[32 * i : 32 * i + 1, :])
            nc.tensor.matmul(
                zps[(h, half2)][0:HD, :],
                lhsT=ones1, rhs=rr, start=True, stop=True, skip_group_check=True,
            )
        # mul order: both halves-0 first so the proj k=5 steps for the first
        # query tiles unblock after two muls
        for half2 in (0, 1):
            for h in (h0, h0 + 1):
                row = (h % 2) * HD
                oX = osbs.pop((h, half2))
                nc.vector.tensor_tensor(
                    oT[j][row : row + HD, half2 * 512 : (half2 + 1) * 512],
                    oX[0:HD, :],
                    zps[(h, half2)][0:HD, :],
                    mybir.AluOpType.mult,
                )

    # 3-deep software pipeline: the PE runs S(t) three chunk-groups ahead of
    # PV(t-3), so the exp latency plus semaphore hops hide behind PE work
    inserts = {
        1: lambda: qkT_job(1), 4: lambda: qkT_job(7),
        7: lambda: qkT_job(2), 10: lambda: qkT_job(8),
        13: lambda: qkT_job(3), 16: lambda: qkT_job(9),
        19: lambda: qkT_job(4), 22: lambda: qkT_job(10),
        26: lambda: qkT_job(5), 29: lambda: qkT_job(11),
        24: lambda: wp_issue(0), 25: lambda: wp_issue(1),
        32: lambda: wp_cast(0), 33: lambda: wp_issue(2),
        35: lambda: wp_cast(1), 37: lambda: wp_issue(3),
        39: lambda: wp_cast(2), 41: lambda: wp_issue(4),
        43: lambda: wp_cast(3), 45: lambda: wp_issue(5),
        47: lambda: wp_cast(4), 49: lambda: wp_cast(5),
    }
    DEPTH = 3
    for t in range(min(DEPTH, T)):
        emit_s(t)
        emit_exp(t)
    for t in range(DEPTH, T):
        emit_s(t)
        emit_exp(t)
        emit_o(t - DEPTH)
        if t - DEPTH in inserts:
            inserts[t - DEPTH]()
    for t in range(T - DEPTH, T):
        emit_o(t)

    # ---------------- proj (tail, PSUM-accumulated) ----------------
    # Pipelined so each tile's k=0..4 accumulation runs ahead of the k=5
    # step (which waits on the last pair's normalization). 'm' tiles use the
    # 3-deep S-half slots, 'o' tiles the freed oaug banks: 4 tiles in flight.
    def proj_head(i, kind):
        if kind == "o":
            psA = att_psum.tile([P, 512], F32, tag="oaug", name="pjA", bufs=2)
            psB = att_psum.tile([P, 256], F32, tag="oaug", name="pjB", bufs=2)
        else:
            psA = sa_tile("pjA")
            psB = sb_tile("pjB")[:, 0:256]
        for k in range(DC - 1):
            for ps_, c0, cw in ((psA, 0, 512), (psB, 512, 256)):
                nc.tensor.matmul(
                    ps_,
                    lhsT=oT[k][:, i * P : (i + 1) * P],
                    rhs=wp[k][:, c0 : c0 + cw],
                    start=(k == 0),
                    stop=False,
                    skip_group_check=True,
                )
        return kind, psA, psB

    def proj_tail(i, h):
        kind, psA, psB = h
        for ps_, c0, cw in ((psA, 0, 512), (psB, 512, 256)):
            nc.tensor.matmul(
                ps_,
                lhsT=oT[DC - 1][:, i * P : (i + 1) * P],
                rhs=wp[DC - 1][:, c0 : c0 + cw],
                start=False,
                stop=True,
                skip_group_check=True,
            )
        yt = att.tile([P, D], F32, tag="y", name="ytile", bufs=4)
        nc.vector.tensor_add(yt[:, 0:512], psA, brep[:, 0:512])
        nc.vector.tensor_add(yt[:, 512:D], psB, brep[:, 512:D])
        # y stores spread over all three DMA queues: a single queue at
        # ~160GB/s would pace the last ~19us of the tail with the 3MB output
        if i < NT - 2:
            nc.scalar.dma_start(out=y[i * P : (i + 1) * P, 0:512], in_=yt[:, 0:512])
            nc.sync.dma_start(out=y[i * P : (i + 1) * P, 512:D], in_=yt[:, 512:D])
        else:
            nc.scalar.dma_start(out=y[i * P : (i + 1) * P, 0:256], in_=yt[:, 0:256])
            nc.sync.dma_start(out=y[i * P : (i + 1) * P, 256:512], in_=yt[:, 256:512])
            nc.gpsimd.dma_start(out=y[i * P : (i + 1) * P, 512:D], in_=yt[:, 512:D])

    kinds = {0: "m", 1: "m", 2: "m", 3: "o"}
    heads = {i: proj_head(i, kinds[i]) for i in range(4)}
    for i in range(NT):
        proj_tail(i, heads.pop(i))
        if i + 4 < NT:
            heads[i + 4] = proj_head(i + 4, kinds[i])


def build_nc(debug: bool = False):
    nc = bacc.Bacc("TRN2", target_bir_lowering=False, debug=debug, enable_asserts=False)
    x = nc.dram_tensor("x", [N, D], F32, kind="ExternalInput").ap()
    w_qkv = nc.dram_tensor("w_qkv", [D, 3 * D], F32, kind="ExternalInput").ap()
    w_proj = nc.dram_tensor("w_proj", [D, D], F32, kind="ExternalInput").ap()
    b_proj = nc.dram_tensor("b_proj", [D], F32, kind="ExternalInput").ap()
    y = nc.dram_tensor("y", [N, D], F32, kind="ExternalOutput").ap()
    with tile.TileContext(nc) as tc:
        with ExitStack() as ctx:
            build_attention(ctx, tc, x, w_qkv, w_proj, b_proj, y)
    nc.compile()
    return nc


_NC = None


def _get_nc():
    global _NC
    if _NC is None:
        _NC = build_nc()
    return _NC


def kernel(inputs, w_qkv, w_proj, b_proj, _trace=False, **run_kwargs):
    from concourse.bass_utils import run_bass_kernel_spmd

    nc = _get_nc()
    inputs = np.asarray(inputs, dtype=np.float32)
    w_qkv = np.ascontiguousarray(np.asarray(w_qkv, dtype=np.float32))
    w_proj = np.ascontiguousarray(np.asarray(w_proj, dtype=np.float32))
    b_proj = np.ascontiguousarray(np.asarray(b_proj, dtype=np.float32))
    in_maps = [
        {
            "x": np.ascontiguousarray(inputs[i]),
            "w_qkv": w_qkv,
            "w_proj": w_proj,
            "b_proj": b_proj,
        }
        for i in range(NCORES)
    ]
    res = run_bass_kernel_spmd(nc, in_maps, list(range(NCORES)), trace=_trace, **run_kwargs)
    out = np.stack([res.results[i]["y"] for i in range(NCORES)], axis=0)
    if _trace:
        return out, res
    return out


# revision 40
# speedup vs baseline: 1.0026x; 1.0026x over previous
"""Multi-head attention forward on 8 Trainium2 NeuronCores.

Problem: nn_Attention_89060441850459
  inputs [8, 1024, 768] f32, w_qkv [768, 2304], w_proj [768, 768], b_proj [768]
  out = proj(softmax(q k^T / sqrt(64)) v) + b_proj,  H=12 heads, hd=64

Sharding: data parallel over batch — each of the 8 cores computes one batch
element end-to-end; weights replicated. No collectives.

Per-core dataflow (fp16 matmul operands, fp32 PSUM accumulation):

  1. xT[d, n]   = PE-transpose of x[n, d]                       (d-major x)
  2. v[n, c]    = x @ w_qkv[:, 1536:]          (s-major, heads padded with a
                  ones-column per head -> [1024, 12*65] so the PV matmul also
                  produces the softmax denominator for free)
  3. qkT[m, n]  = w_qkv[:, :1536].T @ xT      (q/k head-dim-major: [1536, 1024])
  4. per head-PAIR p, per (qpos-half n2, key-chunk m):
       S^T halves of the two heads -> two [128,512] 1-bank PSUM tiles via
         row-tiled matmuls that run concurrently in the PE array
       E = exp(S^T / 8): head a exact exp on ACT, head b f16-Schraudolph on
         the DVE (fused mult+add f32->i16 convert writing f16 exp2 bit
         patterns, +-3% sawtooth; each head's softmax is uniformly exact or
         approximate so the bias normalizes out; end-to-end rel err ~1e-2
         vs the 2e-2 gate)
       O_aug[65, 512] += v_pad_m[:, h].T @ E-half  (PSUM-accumulated over m;
                                                    row 64 = sum_k E = Z)
     then O^T_h = O_aug[0:64] * broadcast(1/Z). Pairs 0-4: reciprocal on a
       [128,8] reshape via DRAM bounces (latency hidden under the window),
       muls on GPSIMD. Last pair: zero-DMA path — 1/Z = exp(-ln Z) on ACT
       (combined exp+ln table set loaded manually at t=0), rank-1 PE matmul
       broadcast, muls on DVE — keeps the PE from idling into a HAM
       re-throttle before the proj tail.
  5. y = O^T-stacked.T @ w_proj + b_proj (PSUM-accumulated tail).

  Scheduling: a PE warmup burst at t=0 (hidden under the input DMA) trips
  the HAM clock gate to 2.4 GHz; x is DMAed on two queues, cast on the DVE,
  and copied out of PSUM one merged 3D copy per chunk. The attention window
  runs a 3-deep software pipeline with THREE 1-bank PSUM slots per S half,
  so the S(t) <- exp(t-3) slot-recycle chain stays off the critical path
  and the window paces at the PE matmul stream.
"""

import sys

if "/opt/trn_rl_repo" not in sys.path:
    sys.path.insert(0, "/opt/trn_rl_repo")

from contextlib import ExitStack

import numpy as np

import concourse.bass as bass
import concourse.mybir as mybir
import concourse.tile as tile
from concourse import bacc
from concourse.masks import make_identity

B, N, D = 8, 1024, 768
H = 12
HD = D // H  # 64
NCORES = 8
P = 128
NT = N // P  # 8 seq chunks
DC = D // P  # 6 d chunks
F32 = mybir.dt.float32
F16 = mybir.dt.float16
I16 = mybir.dt.int16
SCALE = HD**-0.5
# f16 Schraudolph exp2 constants: bits = round(S * A + B), reinterpret i16->f16
EXP_A = float(1024.0 * np.log2(np.e) * SCALE)
EXP_B = float(15 * 1024 - 44.0)
# act_info.json set index of natural_log_exp_and_others (exp AND ln in one set)
ACT_SET_LN_EXP = 6


def build_attention(ctx: ExitStack, tc: "tile.TileContext", x, w_qkv, w_proj, b_proj, y):
    nc = tc.nc
    exp = mybir.ActivationFunctionType.Exp
    ln = mybir.ActivationFunctionType.Ln

    perm = ctx.enter_context(tc.tile_pool(name="perm", bufs=1))
    # PSUM: two 1-bank S-half tags with THREE bufs each (6 banks) + two
    # oaug banks. 3 S slots per half keep the slot-recycle chain
    # (S(t) waits exp(t-3)) two chunks of slack off the PE critical path.
    psum = ctx.enter_context(tc.tile_pool(name="psum", bufs=3, space="PSUM"))
    att_psum = ctx.enter_context(tc.tile_pool(name="attps", bufs=2, space="PSUM"))
    zspill = ctx.enter_context(tc.tile_pool(name="zspill", bufs=2, space="DRAM"))
    tmp = ctx.enter_context(tc.tile_pool(name="tmp", bufs=1))
    att = ctx.enter_context(tc.tile_pool(name="att", bufs=2))

    nc.scalar.add_instruction(
        mybir.InstLoadActFuncSet(
            name=nc.get_next_instruction_name(), ins=[], outs=[],
            act_func_set_id=ACT_SET_LN_EXP,
        )
    )

    def sa_tile(name):
        return psum.tile([P, 512], F32, tag="sa", name=name, bufs=3)

    def sb_tile(name):
        return psum.tile([P, 512], F32, tag="sb", name=name, bufs=3)

    # ---------------- PE warmup (HAM clock gate) ----------------
    # ~15 dummy 512-col matmuls on a zeroed tile keep the PE busy well past
    # the ~3.4us HAM SHORT window while the first x chunks are still in
    # flight, so the lead runs at 2.4 GHz instead of 1.2 (a 9-matmul burst
    # measured as NOT tripping the gate — K=8/8 only arrived at 37us).
    warm16 = perm.tile([P, 512], F16, tag="warm", name="warm16")
    nc.gpsimd.memset(warm16, 0)
    warm_ps = sa_tile("warmps")
    for _ in range(9):
        nc.tensor.matmul(
            warm_ps, lhsT=warm16[:, 0:P], rhs=warm16,
            start=True, stop=True, skip_group_check=True,
        )

    # f32 identity: the x transposes run on the raw f32 input (no pre-cast),
    # the PSUM->SBUF merged copies do the f32->f16 conversion instead
    identity = perm.tile([P, P], F16, tag="identity", name="identity")
    make_identity(nc, identity)
    ones1 = perm.tile([1, HD], F16, tag="ones1", name="ones1")
    nc.vector.memset(ones1, 1.0)

    # persistent SBUF arrays
    qkT = [perm.tile([P, N], F16, tag=f"qkT{m}", name=f"qkT{m}") for m in range(12)]
    vpad = [perm.tile([P, H * (HD + 1)], F16, tag=f"vpad{i}", name=f"vpad{i}") for i in range(NT)]
    oT = [perm.tile([P, N], F16, tag=f"oT{j}", name=f"oT{j}") for j in range(DC)]

    # ---------------- loads, casts, transposes ----------------
    wq = [tmp.tile([P, 3 * D], F16, tag=f"wq{k}", name=f"wq{k}") for k in range(DC)]
    wp = [att.tile([P, D], F16, tag=f"wp{k}", name=f"wp{k}", bufs=1) for k in range(DC)]
    # all six d-chunks of xT in one tile so each x chunk needs ONE psum->sbuf
    # copy (strided 3D dest) instead of six
    xT_all = tmp.tile([P, DC * N], F16, tag="xT", name="xT")
    xT3 = xT_all.rearrange("p (j n) -> p j n", n=N)
    xT = [xT_all[:, j * N : (j + 1) * N] for j in range(DC)]
    xin = ctx.enter_context(tc.tile_pool(name="xin", bufs=3))
    # x first, split across the sync and gpsimd DMA queues. The transposes
    # run on the raw f32 chunks (four d-blocks into a 1-bank sa tile, two
    # into sb); the two merged 3D copies convert f32->f16 on the way out.
    for i in range(NT):
        xt = xin.tile([P, D], F32, tag="x", name="xt", bufs=3)
        eng = nc.sync if i % 2 == 0 else nc.gpsimd
        eng.dma_start(out=xt, in_=x[i * P : (i + 1) * P, :])
        xt16 = xin.tile([P, D], F16, tag="x16", name="xt16", bufs=2)
        nc.vector.tensor_copy(xt16, xt)
        pt = sa_tile("tps")
        pt16 = pt.bitcast(F16)
        for j in range(DC):
            nc.tensor.transpose(
                pt16[:, j * P : (j + 1) * P], xt16[:, j * P : (j + 1) * P], identity
            )
        nc.vector.tensor_copy(
            xT3[:, :, i * P : (i + 1) * P],
            pt16[:, 0 : DC * P].rearrange("p (j c) -> p j c", c=P),
        )

    # weights arrive f32 and DMA cannot cast: stage through f32 tiles and
    # cast on the scalar engine. wv split scalar/sync (it gates the v
    # stream), wqk 3-way — the pre-window DMA (x 3MB + wv 2.3MB + wqk
    # 4.6MB) is the lead's hard floor, so balance the queues at ~3MB each.
    for k in range(DC):
        w32v = xin.tile([P, D], F32, tag="wp32", name="w32v", bufs=2)
        nc.scalar.dma_start(out=w32v, in_=w_qkv[k * P : (k + 1) * P, 2 * D : 3 * D])
        nc.scalar.copy(wq[k][:, 2 * D : 3 * D], w32v)
    for k in range(DC):
        w32qk = xin.tile([P, 2 * D], F32, tag="w32", name="w32qk", bufs=3)
        eng = nc.gpsimd if k % 2 == 0 else nc.scalar
        eng.dma_start(out=w32qk, in_=w_qkv[k * P : (k + 1) * P, 0 : 2 * D])
        nc.scalar.copy(wq[k][:, 0 : 2 * D], w32qk)
    brep = att.tile([P, D], F32, tag="brep", name="brep", bufs=1)
    nc.sync.dma_start(out=brep, in_=b_proj.partition_broadcast(P))

    # bridge burst: the PE sits idle ~20-28us waiting on the wv casts and
    # the HAM clock gate re-throttles to 1.2 GHz right as v starts. These
    # dummy matmuls have no dependencies, so they execute exactly in that
    # hole and keep the array warm (they cost nothing when v is ready).
    bridge_ps = att_psum.tile([P, N // 2], F32, tag="oaug", name="bridge", bufs=2)
    for _ in range(32):
        nc.tensor.matmul(
            bridge_ps, lhsT=warm16[:, 0:P], rhs=warm16,
            start=True, stop=True, skip_group_check=True,
        )

    # ---------------- v then qkT (dense PE lead) ----------------
    # v[i][n, c] = sum_k x[n, k] w_qkv[k, 1536+c], head-padded with a
    # per-head ones column (so the PV matmul also produces the softmax Z)
    for i in range(NT):
        vA = sa_tile("vA")
        vB = sb_tile("vB")
        for k in range(DC):
            for ps_, c0, cw in ((vA, 0, 512), (vB, 512, 256)):
                nc.tensor.matmul(
                    ps_[:, 0:cw],
                    lhsT=xT[k][:, i * P : (i + 1) * P],
                    rhs=wq[k][:, 2 * D + c0 : 2 * D + c0 + cw],
                    start=(k == 0),
                    stop=(k == DC - 1),
                    skip_group_check=True,
                )
        vp3 = vpad[i].rearrange("p (h c) -> p h c", c=HD + 1)
        nc.vector.tensor_copy(
            vp3[:, 0:8, 0:HD], vA.rearrange("p (h c) -> p h c", c=HD)
        )
        nc.vector.tensor_copy(
            vp3[:, 8:12, 0:HD], vB[:, 0:256].rearrange("p (h c) -> p h c", c=HD)
        )
        nc.vector.tensor_scalar(
            vp3[:, :, HD : HD + 1], vp3[:, :, 0:1], 0.0, 1.0,
            mybir.AluOpType.mult, mybir.AluOpType.add,
        )

    # qkT[m][dm, n] = sum_k w_qkv[k, m*128+dm] * xT[k, n]; psum->sbuf copies
    # on the DVE (the scalar queue is busy with the bandwidth-gated wqk
    # casts). Only m=0 and m=6 are built before the attention stream starts;
    # the other ten jobs interleave into the early window, where the PE is
    # the pacer and the exp engines have slack.
    def qkT_job(m):
        qA = sa_tile("qA")
        qB = sb_tile("qB")
        for k in range(DC):
            for ps_, n2 in ((qA, 0), (qB, 1)):
                nc.tensor.matmul(
                    ps_,
                    lhsT=wq[k][:, m * P : (m + 1) * P],
                    rhs=xT[k][:, n2 * 512 : (n2 + 1) * 512],
                    start=(k == 0),
                    stop=(k == DC - 1),
                    skip_group_check=True,
                )
        nc.vector.tensor_copy(qkT[m][:, 0:512], qA)
        nc.vector.tensor_copy(qkT[m][:, 512:1024], qB)

    for m in (0, 6):
        qkT_job(m)

    # wp loads/casts also ride inside the window (needed only at the proj
    # tail); issue and cast are separate hooks so a cast never waits on
    # in-flight DMA data while blocking the scalar queue's exp stream
    wp32s = {}

    def wp_issue(k):
        wp32s[k] = xin.tile([P, D], F32, tag="wp32", name="wp32", bufs=2)
        nc.sync.dma_start(out=wp32s[k], in_=w_proj[k * P : (k + 1) * P, :])

    def wp_cast(k):
        nc.scalar.copy(wp[k], wp32s.pop(k))

    # ---------------- attention ----------------
    # Head PAIRS (heads 2p, 2p+1 share the qkT pair tile: head a on
    # partitions 0:64, head b on 64:128). A chunk is (pair, qpos-half n2,
    # key-block m) — n2 OUTER so only two oaug accumulators live at a time
    # (2 PSUM banks) and boundary frees hide under the stream.
    chunks = [(p, n2, m) for p in range(H // 2) for n2 in range(2) for m in range(NT)]
    T = len(chunks)
    oaug = {}
    sps = {}
    epool = {}

    def emit_s(t):
        p, n2, m = chunks[t]
        spa = sa_tile("spa")
        spb = sb_tile("spb")
        sps[t] = (spa, spb)
        for half, sp in ((0, spa), (1, spb)):
            row = half * HD
            kT_h = qkT[6 + p][row : row + HD, :]
            qT_h = qkT[p][row : row + HD, :]
            nc.tensor.matmul(
                sp,
                lhsT=kT_h[:, m * P : (m + 1) * P],
                rhs=qT_h[:, n2 * 512 : (n2 + 1) * 512],
                start=True,
                stop=True,
            )

    def emit_exp(t):
        # half-split: head a exact exp on ACT, head b f16-Schraudolph on the
        # DVE — halves the exp latency per chunk and keeps each head's
        # softmax uniformly exact or approximate
        spa, spb = sps.pop(t)
        ea = att.tile([P, 512], F16, tag="ea", name="ea", bufs=5)
        eb = att.tile([P, 512], F16, tag="eb", name="eb", bufs=5)
        epool[t] = (ea, eb)
        nc.scalar.activation(ea, spa, exp, scale=SCALE)
        nc.vector.tensor_scalar(
            eb.bitcast(I16), spb, EXP_A, EXP_B,
            mybir.AluOpType.mult, mybir.AluOpType.add,
        )

    def emit_o(t):
        p, n2, m = chunks[t]
        if m == 0:
            # lazy alloc at the PV stage so the wait on the outgoing half's
            # osb copies never gets ahead of the PE stream
            for h in (2 * p, 2 * p + 1):
                oaug[(h, n2)] = att_psum.tile(
                    [HD + 1, N // 2], F32, tag="oaug", name="oaug", bufs=2
                )
        ea, eb = epool.pop(t)
        for half, e in ((0, ea), (1, eb)):
            h = 2 * p + half
            vl = vpad[m][:, h * (HD + 1) : (h + 1) * (HD + 1)]
            nc.tensor.matmul(
                oaug[(h, n2)],
                lhsT=vl,
                rhs=e,
                start=(m == 0),
                stop=(m == NT - 1),
                skip_group_check=True,
            )
        if m == NT - 1:
            emit_osb(2 * p, n2)
            emit_osb(2 * p + 1, n2)
            if n2 == 1:
                if p == H // 2 - 1:
                    emit_norm_fast(2 * p)
                else:
                    emit_norm(2 * p)
                    emit_norm(2 * p + 1)

    osbs = {}

    def emit_osb(h, half2):
        # Copy O-half + its Z row to SBUF (frees one PSUM bank). Head b's
        # copy goes to the scalar engine so both heads copy in parallel.
        oa = oaug.pop((h, half2))
        osb = att.tile([HD + 1, N // 2], F32, tag="osb", name="osb", bufs=8)
        if h % 2 == 0:
            nc.vector.tensor_copy(osb, oa)
        else:
            nc.scalar.copy(osb, oa)
        osbs[(h, half2)] = osb

    def emit_norm(h):
        # pairs 0-4: reciprocal on a [128,8] reshape via DRAM bounces; the
        # chain latency hides under the remaining attention window. The
        # final muls run on GPSIMD (SBUF-only engine, otherwise idle).
        row = (h % 2) * HD
        oA = osbs.pop((h, 0))
        oB = osbs.pop((h, 1))
        zd = zspill.tile([1, N], F32, tag="zd", name="zd", bufs=2)
        nc.sync.dma_start(out=zd[0:1, 0 : N // 2], in_=oA[HD : HD + 1, :])
        nc.sync.dma_start(out=zd[0:1, N // 2 : N], in_=oB[HD : HD + 1, :])
        z8 = att.tile([P, N // P], F32, tag="z8", name="z8")
        nc.sync.dma_start(out=z8, in_=zd.rearrange("o (p f) -> (o p) f", p=P))
        r8 = att.tile([P, N // P], F32, tag="r8", name="r8")
        nc.vector.reciprocal(r8, z8)
        rd = zspill.tile([1, N], F32, tag="rd", name="rd", bufs=2)
        nc.sync.dma_start(out=rd.rearrange("o (p f) -> (o p) f", p=P), in_=r8)
        zrep = att.tile([HD, N], F32, tag="zrep", name="zrep")
        nc.sync.dma_start(out=zrep, in_=rd[0, :].partition_broadcast(HD))
        nc.gpsimd.tensor_tensor(
            oT[h // 2][row : row + HD, 0 : N // 2], oA[0:HD, :], zrep[:, 0 : N // 2],
            mybir.AluOpType.mult,
        )
        nc.gpsimd.tensor_tensor(
            oT[h // 2][row : row + HD, N // 2 : N], oB[0:HD, :], zrep[:, N // 2 : N],
            mybir.AluOpType.mult,
        )

    def emit_norm_fast(h0):
        # last pair: zero-DMA normalization so the PE flows straight into the
        # proj tail. 1/Z = exp(-ln Z) on ACT (one combined table set), rank-1
        # PE matmuls broadcast it to [64,512], muls on the DVE.
        j = h0 // 2
        quads = [(h0, 0), (h0, 1), (h0 + 1, 0), (h0 + 1, 1)]
        # engine partition offsets must be 32-aligned: space the 4 Z rows at
        # partitions 0/32/64/96
        z4 = att.tile([97, N // 2], F32, tag="z4", name="z4")
        for i, (h, half2) in enumerate(quads):
            nc.vector.tensor_copy(
                z4[32 * i : 32 * i + 1, :], osbs[(h, half2)][HD : HD + 1, :]
            )
        l4 = att.tile([97, N // 2], F32, tag="l4", name="l4")
        nc.scalar.activation(l4, z4, ln)
        r4 = att.tile([97, N // 2], F16, tag="r4", name="r4")
        nc.scalar.activation(r4, l4, exp, scale=-1.0)
        zps = {}
        for i, (h, half2) in enumerate(quads):
            zps[(h, half2)] = sa_tile("zps") if half2 == 0 else sb_tile("zps")
            rr = att.tile([1, N // 2], F16, tag="rrow", name="rrow", bufs=4)
            nc.vector.tensor_copy(rr, r4[32 * i : 32 * i + 1, :])
            nc.tensor.matmul(
                zps[(h, half2)][0:HD, :],
                lhsT=ones1, rhs=rr, start=True, stop=True, skip_group_check=True,
            )
        # mul order: both halves-0 first so the proj k=5 steps for the first
        # query tiles unblock after two muls
        for half2 in (0, 1):
            for h in (h0, h0 + 1):
                row = (h % 2) * HD
                oX = osbs.pop((h, half2))
                nc.vector.tensor_tensor(
                    oT[j][row : row + HD, half2 * 512 : (half2 + 1) * 512],
                    oX[0:HD, :],
                    zps[(h, half2)][0:HD, :],
                    mybir.AluOpType.mult,
                )

    # 3-deep software pipeline: the PE runs S(t) three chunk-groups ahead of
    # PV(t-3), so the exp latency plus semaphore hops hide behind PE work
    inserts = {
        1: lambda: qkT_job(1), 4: lambda: qkT_job(7),
        7: lambda: qkT_job(2), 10: lambda: qkT_job(8),
        13: lambda: qkT_job(3), 16: lambda: qkT_job(9),
        19: lambda: qkT_job(4), 22: lambda: qkT_job(10),
        26: lambda: qkT_job(5), 29: lambda: qkT_job(11),
        24: lambda: wp_issue(0), 25: lambda: wp_issue(1),
        32: lambda: wp_cast(0), 33: lambda: wp_issue(2),
        35: lambda: wp_cast(1), 37: lambda: wp_issue(3),
        39: lambda: wp_cast(2), 41: lambda: wp_issue(4),
        43: lambda: wp_cast(3), 45: lambda: wp_issue(5),
        47: lambda: wp_cast(4), 49: lambda: wp_cast(5),
    }
    DEPTH = 3
    for t in range(min(DEPTH, T)):
        emit_s(t)
        emit_exp(t)
    for t in range(DEPTH, T):
        emit_s(t)
        emit_exp(t)
        emit_o(t - DEPTH)
        if t - DEPTH in inserts:
            inserts[t - DEPTH]()
    for t in range(T - DEPTH, T):
        emit_o(t)

    # ---------------- proj (tail, PSUM-accumulated) ----------------
    # Pipelined so each tile's k=0..4 accumulation runs ahead of the k=5
    # step (which waits on the last pair's normalization). 'm' tiles use the
    # 3-deep S-half slots, 'o' tiles the freed oaug banks: 4 tiles in flight.
    def proj_head(i, kind):
        if kind == "o":
            psA = att_psum.tile([P, 512], F32, tag="oaug", name="pjA", bufs=2)
            psB = att_psum.tile([P, 256], F32, tag="oaug", name="pjB", bufs=2)
        else:
            psA = sa_tile("pjA")
            psB = sb_tile("pjB")[:, 0:256]
        for k in range(DC - 1):
            for ps_, c0, cw in ((psA, 0, 512), (psB, 512, 256)):
                nc.tensor.matmul(
                    ps_,
                    lhsT=oT[k][:, i * P : (i + 1) * P],
                    rhs=wp[k][:, c0 : c0 + cw],
                    start=(k == 0),
                    stop=False,
                    skip_group_check=True,
                )
        return kind, psA, psB

    def proj_tail(i, h):
        kind, psA, psB = h
        for ps_, c0, cw in ((psA, 0, 512), (psB, 512, 256)):
            nc.tensor.matmul(
                ps_,
                lhsT=oT[DC - 1][:, i * P : (i + 1) * P],
                rhs=wp[DC - 1][:, c0 : c0 + cw],
                start=False,
                stop=True,
                skip_group_check=True,
            )
        yt = att.tile([P, D], F32, tag="y", name="ytile", bufs=4)
        nc.vector.tensor_add(yt[:, 0:512], psA, brep[:, 0:512])
        nc.vector.tensor_add(yt[:, 512:D], psB, brep[:, 512:D])
        # y stores spread over all three DMA queues: a single queue at
        # ~160GB/s would pace the last ~19us of the tail with the 3MB output
        if i < NT - 2:
            nc.scalar.dma_start(out=y[i * P : (i + 1) * P, 0:512], in_=yt[:, 0:512])
            nc.sync.dma_start(out=y[i * P : (i + 1) * P, 512:D], in_=yt[:, 512:D])
        else:
            nc.scalar.dma_start(out=y[i * P : (i + 1) * P, 0:256], in_=yt[:, 0:256])
            nc.sync.dma_start(out=y[i * P : (i + 1) * P, 256:512], in_=yt[:, 256:512])
            nc.gpsimd.dma_start(out=y[i * P : (i + 1) * P, 512:D], in_=yt[:, 512:D])

    kinds = {0: "m", 1: "m", 2: "m", 3: "o"}
    heads = {i: proj_head(i, kinds[i]) for i in range(4)}
    for i in range(NT):
        proj_tail(i, heads.pop(i))
        if i + 4 < NT:
            heads[i + 4] = proj_head(i + 4, kinds[i])


def build_nc(debug: bool = False):
    nc = bacc.Bacc("TRN2", target_bir_lowering=False, debug=debug, enable_asserts=False)
    x = nc.dram_tensor("x", [N, D], F32, kind="ExternalInput").ap()
    w_qkv = nc.dram_tensor("w_qkv", [D, 3 * D], F32, kind="ExternalInput").ap()
    w_proj = nc.dram_tensor("w_proj", [D, D], F32, kind="ExternalInput").ap()
    b_proj = nc.dram_tensor("b_proj", [D], F32, kind="ExternalInput").ap()
    y = nc.dram_tensor("y", [N, D], F32, kind="ExternalOutput").ap()
    with tile.TileContext(nc) as tc:
        with ExitStack() as ctx:
            build_attention(ctx, tc, x, w_qkv, w_proj, b_proj, y)
    nc.compile()
    return nc


_NC = None


def _get_nc():
    global _NC
    if _NC is None:
        _NC = build_nc()
    return _NC


def kernel(inputs, w_qkv, w_proj, b_proj, _trace=False, **run_kwargs):
    from concourse.bass_utils import run_bass_kernel_spmd

    nc = _get_nc()
    inputs = np.asarray(inputs, dtype=np.float32)
    w_qkv = np.ascontiguousarray(np.asarray(w_qkv, dtype=np.float32))
    w_proj = np.ascontiguousarray(np.asarray(w_proj, dtype=np.float32))
    b_proj = np.ascontiguousarray(np.asarray(b_proj, dtype=np.float32))
    in_maps = [
        {
            "x": np.ascontiguousarray(inputs[i]),
            "w_qkv": w_qkv,
            "w_proj": w_proj,
            "b_proj": b_proj,
        }
        for i in range(NCORES)
    ]
    res = run_bass_kernel_spmd(nc, in_maps, list(range(NCORES)), trace=_trace, **run_kwargs)
    out = np.stack([res.results[i]["y"] for i in range(NCORES)], axis=0)
    if _trace:
        return out, res
    return out


# revision 41
# speedup vs baseline: 1.1814x; 1.1783x over previous
"""Multi-head attention forward on 8 Trainium2 NeuronCores.

Problem: nn_Attention_89060441850459
  inputs [8, 1024, 768] f32, w_qkv [768, 2304], w_proj [768, 768], b_proj [768]
  out = proj(softmax(q k^T / sqrt(64)) v) + b_proj,  H=12 heads, hd=64

Sharding: data parallel over batch — each of the 8 cores computes one batch
element end-to-end; weights replicated. No collectives.

Per-core dataflow (fp16 matmul operands, fp32 PSUM accumulation):

  1. xT[d, n]   = PE-transpose of x[n, d]                       (d-major x)
  2. v[n, c]    = x @ w_qkv[:, 1536:]          (s-major, heads padded with a
                  ones-column per head -> [1024, 12*65] so the PV matmul also
                  produces the softmax denominator for free)
  3. qkT[m, n]  = w_qkv[:, :1536].T @ xT      (q/k head-dim-major: [1536, 1024])
  4. per head-PAIR p, per (qpos-half n2, key-chunk m):
       S^T halves of the two heads -> two [128,512] 1-bank PSUM tiles via
         row-tiled matmuls that run concurrently in the PE array
       E = exp(S^T / 8): head a exact exp on ACT, head b f16-Schraudolph on
         the DVE (fused mult+add f32->i16 convert writing f16 exp2 bit
         patterns, +-3% sawtooth; each head's softmax is uniformly exact or
         approximate so the bias normalizes out; end-to-end rel err ~1e-2
         vs the 2e-2 gate)
       O_aug[65, 512] += v_pad_m[:, h].T @ E-half  (PSUM-accumulated over m;
                                                    row 64 = sum_k E = Z)
     then O^T_h = O_aug[0:64] * broadcast(1/Z). Pairs 0-4: reciprocal on a
       [128,8] reshape via DRAM bounces (latency hidden under the window),
       muls on GPSIMD. Last pair: zero-DMA path — 1/Z = exp(-ln Z) on ACT
       (combined exp+ln table set loaded manually at t=0), rank-1 PE matmul
       broadcast, muls on DVE — keeps the PE from idling into a HAM
       re-throttle before the proj tail.
  5. y = O^T-stacked.T @ w_proj + b_proj (PSUM-accumulated tail).

  Scheduling: a PE warmup burst at t=0 (hidden under the input DMA) trips
  the HAM clock gate to 2.4 GHz; x is DMAed on two queues, cast on the DVE,
  and copied out of PSUM one merged 3D copy per chunk. The attention window
  runs a 3-deep software pipeline with THREE 1-bank PSUM slots per S half,
  so the S(t) <- exp(t-3) slot-recycle chain stays off the critical path
  and the window paces at the PE matmul stream.
"""

import sys

if "/opt/trn_rl_repo" not in sys.path:
    sys.path.insert(0, "/opt/trn_rl_repo")

from contextlib import ExitStack

import numpy as np

import concourse.bass as bass
import concourse.mybir as mybir
import concourse.tile as tile
from concourse import bacc
from concourse.masks import make_identity

B, N, D = 8, 1024, 768
H = 12
HD = D // H  # 64
NCORES = 8
P = 128
NT = N // P  # 8 seq chunks
DC = D // P  # 6 d chunks
F32 = mybir.dt.float32
F16 = mybir.dt.float16
I16 = mybir.dt.int16
SCALE = HD**-0.5
# f16 Schraudolph exp2 constants: bits = round(S * A + B), reinterpret i16->f16
EXP_A = float(1024.0 * np.log2(np.e) * SCALE)
EXP_B = float(15 * 1024 - 44.0)
# act_info.json set index of natural_log_exp_and_others (exp AND ln in one set)
ACT_SET_LN_EXP = 6


def build_attention(ctx: ExitStack, tc: "tile.TileContext", x, w_qkv, w_proj, b_proj, y):
    nc = tc.nc
    exp = mybir.ActivationFunctionType.Exp
    ln = mybir.ActivationFunctionType.Ln

    perm = ctx.enter_context(tc.tile_pool(name="perm", bufs=1))
    # PSUM: two 1-bank S-half tags with THREE bufs each (6 banks) + two
    # oaug banks. 3 S slots per half keep the slot-recycle chain
    # (S(t) waits exp(t-3)) two chunks of slack off the PE critical path.
    psum = ctx.enter_context(tc.tile_pool(name="psum", bufs=3, space="PSUM"))
    att_psum = ctx.enter_context(tc.tile_pool(name="attps", bufs=2, space="PSUM"))
    zspill = ctx.enter_context(tc.tile_pool(name="zspill", bufs=2, space="DRAM"))
    tmp = ctx.enter_context(tc.tile_pool(name="tmp", bufs=1))
    att = ctx.enter_context(tc.tile_pool(name="att", bufs=2))

    nc.scalar.add_instruction(
        mybir.InstLoadActFuncSet(
            name=nc.get_next_instruction_name(), ins=[], outs=[],
            act_func_set_id=ACT_SET_LN_EXP,
        )
    )

    def sa_tile(name):
        return psum.tile([P, 512], F32, tag="sa", name=name, bufs=3)

    def sb_tile(name):
        return psum.tile([P, 512], F32, tag="sb", name=name, bufs=3)

    # ---------------- PE warmup (HAM clock gate) ----------------
    # ~15 dummy 512-col matmuls on a zeroed tile keep the PE busy well past
    # the ~3.4us HAM SHORT window while the first x chunks are still in
    # flight, so the lead runs at 2.4 GHz instead of 1.2 (a 9-matmul burst
    # measured as NOT tripping the gate — K=8/8 only arrived at 37us).
    warm16 = perm.tile([P, 512], F16, tag="warm", name="warm16")
    nc.gpsimd.memset(warm16, 0)
    warm_ps = sa_tile("warmps")
    for _ in range(9):
        nc.tensor.matmul(
            warm_ps, lhsT=warm16[:, 0:P], rhs=warm16,
            start=True, stop=True, skip_group_check=True,
        )

    # f32 identity: the x transposes run on the raw f32 input (no pre-cast),
    # the PSUM->SBUF merged copies do the f32->f16 conversion instead
    identity = perm.tile([P, P], F16, tag="identity", name="identity")
    make_identity(nc, identity)
    ones1 = perm.tile([1, HD], F16, tag="ones1", name="ones1")
    nc.vector.memset(ones1, 1.0)

    # persistent SBUF arrays
    qkT = [perm.tile([P, N], F16, tag=f"qkT{m}", name=f"qkT{m}") for m in range(12)]
    vpad = [perm.tile([P, H * (HD + 1)], F16, tag=f"vpad{i}", name=f"vpad{i}") for i in range(NT)]
    oT = [perm.tile([P, N], F16, tag=f"oT{j}", name=f"oT{j}") for j in range(DC)]

    # ---------------- loads, casts, transposes ----------------
    wq = [tmp.tile([P, 3 * D], F16, tag=f"wq{k}", name=f"wq{k}") for k in range(DC)]
    wp = [att.tile([P, D], F16, tag=f"wp{k}", name=f"wp{k}", bufs=1) for k in range(DC)]
    # all six d-chunks of xT in one tile so each x chunk needs ONE psum->sbuf
    # copy (strided 3D dest) instead of six
    xT_all = tmp.tile([P, DC * N], F16, tag="xT", name="xT")
    xT3 = xT_all.rearrange("p (j n) -> p j n", n=N)
    xT = [xT_all[:, j * N : (j + 1) * N] for j in range(DC)]
    xin = ctx.enter_context(tc.tile_pool(name="xin", bufs=3))
    # x first, split across the sync and gpsimd DMA queues. The transposes
    # run on the raw f32 chunks (four d-blocks into a 1-bank sa tile, two
    # into sb); the two merged 3D copies convert f32->f16 on the way out.
    for i in range(NT):
        xt = xin.tile([P, D], F32, tag="x", name="xt", bufs=3)
        eng = nc.sync if i % 2 == 0 else nc.gpsimd
        eng.dma_start(out=xt, in_=x[i * P : (i + 1) * P, :])
        xt16 = xin.tile([P, D], F16, tag="x16", name="xt16", bufs=2)
        nc.vector.tensor_copy(xt16, xt)
        pt = sa_tile("tps")
        pt16 = pt.bitcast(F16)
        for j in range(DC):
            nc.tensor.transpose(
                pt16[:, j * P : (j + 1) * P], xt16[:, j * P : (j + 1) * P], identity
            )
        nc.vector.tensor_copy(
            xT3[:, :, i * P : (i + 1) * P],
            pt16[:, 0 : DC * P].rearrange("p (j c) -> p j c", c=P),
        )

    # weights arrive f32 and DMA cannot cast: stage through f32 tiles and
    # cast on the scalar engine. wv split scalar/sync (it gates the v
    # stream), wqk 3-way — the pre-window DMA (x 3MB + wv 2.3MB + wqk
    # 4.6MB) is the lead's hard floor, so balance the queues at ~3MB each.
    for k in range(DC):
        w32v = xin.tile([P, D], F32, tag="wp32", name="w32v", bufs=2)
        nc.scalar.dma_start(out=w32v, in_=w_qkv[k * P : (k + 1) * P, 2 * D : 3 * D])
        nc.scalar.copy(wq[k][:, 2 * D : 3 * D], w32v)
    for k in range(DC):
        w32qk = xin.tile([P, 2 * D], F32, tag="w32", name="w32qk", bufs=3)
        eng = nc.gpsimd if k % 2 == 0 else nc.scalar
        eng.dma_start(out=w32qk, in_=w_qkv[k * P : (k + 1) * P, 0 : 2 * D])
        nc.scalar.copy(wq[k][:, 0 : 2 * D], w32qk)
    brep = att.tile([P, D], F32, tag="brep", name="brep", bufs=1)
    nc.sync.dma_start(out=brep, in_=b_proj.partition_broadcast(P))

    # bridge burst: the PE sits idle ~20-28us waiting on the wv casts and
    # the HAM clock gate re-throttles to 1.2 GHz right as v starts. These
    # dummy matmuls have no dependencies, so they execute exactly in that
    # hole and keep the array warm (they cost nothing when v is ready).
    bridge_ps = att_psum.tile([P, N // 2], F32, tag="oaug", name="bridge", bufs=2)
    for _ in range(32):
        nc.tensor.matmul(
            bridge_ps, lhsT=warm16[:, 0:P], rhs=warm16,
            start=True, stop=True, skip_group_check=True,
        )

    # ---------------- v then qkT (dense PE lead) ----------------
    # v[i][n, c] = sum_k x[n, k] w_qkv[k, 1536+c], head-padded with a
    # per-head ones column (so the PV matmul also produces the softmax Z)
    for i in range(NT):
        vA = sa_tile("vA")
        vB = sb_tile("vB")
        for k in range(DC):
            for ps_, c0, cw in ((vA, 0, 512), (vB, 512, 256)):
                nc.tensor.matmul(
                    ps_[:, 0:cw],
                    lhsT=xT[k][:, i * P : (i + 1) * P],
                    rhs=wq[k][:, 2 * D + c0 : 2 * D + c0 + cw],
                    start=(k == 0),
                    stop=(k == DC - 1),
                    skip_group_check=True,
                )
        vp3 = vpad[i].rearrange("p (h c) -> p h c", c=HD + 1)
        nc.vector.tensor_copy(
            vp3[:, 0:8, 0:HD], vA.rearrange("p (h c) -> p h c", c=HD)
        )
        nc.vector.tensor_copy(
            vp3[:, 8:12, 0:HD], vB[:, 0:256].rearrange("p (h c) -> p h c", c=HD)
        )
        nc.vector.tensor_scalar(
            vp3[:, :, HD : HD + 1], vp3[:, :, 0:1], 0.0, 1.0,
            mybir.AluOpType.mult, mybir.AluOpType.add,
        )

    # qkT[m][dm, n] = sum_k w_qkv[k, m*128+dm] * xT[k, n]; psum->sbuf copies
    # on the DVE (the scalar queue is busy with the bandwidth-gated wqk
    # casts). Only m=0 and m=6 are built before the attention stream starts;
    # the other ten jobs interleave into the early window, where the PE is
    # the pacer and the exp engines have slack.
    def qkT_job(m):
        qA = sa_tile("qA")
        qB = sb_tile("qB")
        for k in range(DC):
            for ps_, n2 in ((qA, 0), (qB, 1)):
                nc.tensor.matmul(
                    ps_,
                    lhsT=wq[k][:, m * P : (m + 1) * P],
                    rhs=xT[k][:, n2 * 512 : (n2 + 1) * 512],
                    start=(k == 0),
                    stop=(k == DC - 1),
                    skip_group_check=True,
                )
        nc.vector.tensor_copy(qkT[m][:, 0:512], qA)
        nc.vector.tensor_copy(qkT[m][:, 512:1024], qB)

    for m in (0, 6):
        qkT_job(m)

    # wp loads/casts also ride inside the window (needed only at the proj
    # tail); issue and cast are separate hooks so a cast never waits on
    # in-flight DMA data while blocking the scalar queue's exp stream
    wp32s = {}

    def wp_issue(k):
        wp32s[k] = xin.tile([P, D], F32, tag="wp32", name="wp32", bufs=2)
        nc.sync.dma_start(out=wp32s[k], in_=w_proj[k * P : (k + 1) * P, :])

    def wp_cast(k):
        nc.scalar.copy(wp[k], wp32s.pop(k))

    # ---------------- attention ----------------
    # Head PAIRS (heads 2p, 2p+1 share the qkT pair tile: head a on
    # partitions 0:64, head b on 64:128). A chunk is (pair, qpos-half n2,
    # key-block m) — n2 OUTER so only two oaug accumulators live at a time
    # (2 PSUM banks) and boundary frees hide under the stream.
    chunks = [(p, n2, m) for p in range(H // 2) for n2 in range(2) for m in range(NT)]
    T = len(chunks)
    oaug = {}
    sps = {}
    epool = {}

    def emit_s(t):
        p, n2, m = chunks[t]
        spa = sa_tile("spa")
        spb = sb_tile("spb")
        sps[t] = (spa, spb)
        for half, sp in ((0, spa), (1, spb)):
            row = half * HD
            kT_h = qkT[6 + p][row : row + HD, :]
            qT_h = qkT[p][row : row + HD, :]
            nc.tensor.matmul(
                sp,
                lhsT=kT_h[:, m * P : (m + 1) * P],
                rhs=qT_h[:, n2 * 512 : (n2 + 1) * 512],
                start=True,
                stop=True,
            )

    def emit_exp(t):
        # half-split: head a exact exp on ACT, head b f16-Schraudolph on the
        # DVE — halves the exp latency per chunk and keeps each head's
        # softmax uniformly exact or approximate
        spa, spb = sps.pop(t)
        ea = att.tile([P, 512], F16, tag="ea", name="ea", bufs=6)
        eb = att.tile([P, 512], F16, tag="eb", name="eb", bufs=6)
        epool[t] = (ea, eb)
        nc.scalar.activation(ea, spa, exp, scale=SCALE)
        nc.vector.tensor_scalar(
            eb.bitcast(I16), spb, EXP_A, EXP_B,
            mybir.AluOpType.mult, mybir.AluOpType.add,
        )

    def emit_o(t):
        p, n2, m = chunks[t]
        if m == 0:
            # lazy alloc at the PV stage so the wait on the outgoing half's
            # osb copies never gets ahead of the PE stream
            for h in (2 * p, 2 * p + 1):
                oaug[(h, n2)] = att_psum.tile(
                    [HD + 1, N // 2], F32, tag="oaug", name="oaug", bufs=2
                )
        ea, eb = epool.pop(t)
        for half, e in ((0, ea), (1, eb)):
            h = 2 * p + half
            vl = vpad[m][:, h * (HD + 1) : (h + 1) * (HD + 1)]
            nc.tensor.matmul(
                oaug[(h, n2)],
                lhsT=vl,
                rhs=e,
                start=(m == 0),
                stop=(m == NT - 1),
                skip_group_check=True,
            )
        if m == NT - 1:
            emit_osb(2 * p, n2)
            emit_osb(2 * p + 1, n2)
            if n2 == 1:
                if p == H // 2 - 1:
                    emit_norm_fast(2 * p)
                else:
                    emit_norm(2 * p)
                    emit_norm(2 * p + 1)

    osbs = {}

    def emit_osb(h, half2):
        # Copy O-half + its Z row to SBUF (frees one PSUM bank). Head b's
        # copy goes to the scalar engine so both heads copy in parallel.
        oa = oaug.pop((h, half2))
        osb = att.tile([HD + 1, N // 2], F32, tag="osb", name="osb", bufs=8)
        if h % 2 == 0:
            nc.vector.tensor_copy(osb, oa)
        else:
            nc.scalar.copy(osb, oa)
        osbs[(h, half2)] = osb

    def emit_norm(h):
        # pairs 0-4: reciprocal on a [128,8] reshape via DRAM bounces; the
        # chain latency hides under the remaining attention window. The
        # final muls run on GPSIMD (SBUF-only engine, otherwise idle).
        row = (h % 2) * HD
        oA = osbs.pop((h, 0))
        oB = osbs.pop((h, 1))
        zd = zspill.tile([1, N], F32, tag="zd", name="zd", bufs=2)
        nc.sync.dma_start(out=zd[0:1, 0 : N // 2], in_=oA[HD : HD + 1, :])
        nc.sync.dma_start(out=zd[0:1, N // 2 : N], in_=oB[HD : HD + 1, :])
        z8 = att.tile([P, N // P], F32, tag="z8", name="z8")
        nc.sync.dma_start(out=z8, in_=zd.rearrange("o (p f) -> (o p) f", p=P))
        r8 = att.tile([P, N // P], F32, tag="r8", name="r8")
        nc.vector.reciprocal(r8, z8)
        rd = zspill.tile([1, N], F32, tag="rd", name="rd", bufs=2)
        nc.sync.dma_start(out=rd.rearrange("o (p f) -> (o p) f", p=P), in_=r8)
        zrep = att.tile([HD, N], F32, tag="zrep", name="zrep")
        nc.sync.dma_start(out=zrep, in_=rd[0, :].partition_broadcast(HD))
        nc.gpsimd.tensor_tensor(
            oT[h // 2][row : row + HD, 0 : N // 2], oA[0:HD, :], zrep[:, 0 : N // 2],
            mybir.AluOpType.mult,
        )
        nc.gpsimd.tensor_tensor(
            oT[h // 2][row : row + HD, N // 2 : N], oB[0:HD, :], zrep[:, N // 2 : N],
            mybir.AluOpType.mult,
        )

    def emit_norm_fast(h0):
        # last pair: zero-DMA normalization so the PE flows straight into the
        # proj tail. 1/Z = exp(-ln Z) on ACT (one combined table set), rank-1
        # PE matmuls broadcast it to [64,512], muls on the DVE.
        j = h0 // 2
        quads = [(h0, 0), (h0, 1), (h0 + 1, 0), (h0 + 1, 1)]
        # engine partition offsets must be 32-aligned: space the 4 Z rows at
        # partitions 0/32/64/96
        z4 = att.tile([97, N // 2], F32, tag="z4", name="z4")
        for i, (h, half2) in enumerate(quads):
            nc.vector.tensor_copy(
                z4[32 * i : 32 * i + 1, :], osbs[(h, half2)][HD : HD + 1, :]
            )
        l4 = att.tile([97, N // 2], F32, tag="l4", name="l4")
        nc.scalar.activation(l4, z4, ln)
        r4 = att.tile([97, N // 2], F16, tag="r4", name="r4")
        nc.scalar.activation(r4, l4, exp, scale=-1.0)
        zps = {}
        for i, (h, half2) in enumerate(quads):
            zps[(h, half2)] = sa_tile("zps") if half2 == 0 else sb_tile("zps")
            rr = att.tile([1, N // 2], F16, tag="rrow", name="rrow", bufs=4)
            nc.vector.tensor_copy(rr, r4[32 * i : 32 * i + 1, :])
            nc.tensor.matmul(
                zps[(h, half2)][0:HD, :],
                lhsT=ones1, rhs=rr, start=True, stop=True, skip_group_check=True,
            )
        # mul order: both halves-0 first so the proj k=5 steps for the first
        # query tiles unblock after two muls
        for half2 in (0, 1):
            for h in (h0, h0 + 1):
                row = (h % 2) * HD
                oX = osbs.pop((h, half2))
                nc.vector.tensor_tensor(
                    oT[j][row : row + HD, half2 * 512 : (half2 + 1) * 512],
                    oX[0:HD, :],
                    zps[(h, half2)][0:HD, :],
                    mybir.AluOpType.mult,
                )

    # 3-deep software pipeline: the PE runs S(t) three chunk-groups ahead of
    # PV(t-3), so the exp latency plus semaphore hops hide behind PE work
    inserts = {
        1: lambda: qkT_job(1), 4: lambda: qkT_job(7),
        7: lambda: qkT_job(2), 10: lambda: qkT_job(8),
        13: lambda: qkT_job(3), 16: lambda: qkT_job(9),
        19: lambda: qkT_job(4), 22: lambda: qkT_job(10),
        26: lambda: qkT_job(5), 29: lambda: qkT_job(11),
        24: lambda: wp_issue(0), 25: lambda: wp_issue(1),
        32: lambda: wp_cast(0), 33: lambda: wp_issue(2),
        35: lambda: wp_cast(1), 37: lambda: wp_issue(3),
        39: lambda: wp_cast(2), 41: lambda: wp_issue(4),
        43: lambda: wp_cast(3), 45: lambda: wp_issue(5),
        47: lambda: wp_cast(4), 49: lambda: wp_cast(5),
    }
    DEPTH = 4
    for t in range(min(DEPTH, T)):
        emit_s(t)
        emit_exp(t)
    for t in range(DEPTH, T):
        emit_s(t)
        emit_exp(t)
        emit_o(t - DEPTH)
        if t - DEPTH in inserts:
            inserts[t - DEPTH]()
    for t in range(T - DEPTH, T):
        emit_o(t)

    # ---------------- proj (tail, PSUM-accumulated) ----------------
    # Pipelined so each tile's k=0..4 accumulation runs ahead of the k=5
    # step (which waits on the last pair's normalization). 'm' tiles use the
    # 3-deep S-half slots, 'o' tiles the freed oaug banks: 4 tiles in flight.
    def proj_head(i, kind):
        if kind == "o":
            psA = att_psum.tile([P, 512], F32, tag="oaug", name="pjA", bufs=2)
            psB = att_psum.tile([P, 256], F32, tag="oaug", name="pjB", bufs=2)
        else:
            psA = sa_tile("pjA")
            psB = sb_tile("pjB")[:, 0:256]
        for k in range(DC - 1):
            for ps_, c0, cw in ((psA, 0, 512), (psB, 512, 256)):
                nc.tensor.matmul(
                    ps_,
                    lhsT=oT[k][:, i * P : (i + 1) * P],
                    rhs=wp[k][:, c0 : c0 + cw],
                    start=(k == 0),
                    stop=False,
                    skip_group_check=True,
                )
        return kind, psA, psB

    def proj_tail(i, h):
        kind, psA, psB = h
        for ps_, c0, cw in ((psA, 0, 512), (psB, 512, 256)):
            nc.tensor.matmul(
                ps_,
                lhsT=oT[DC - 1][:, i * P : (i + 1) * P],
                rhs=wp[DC - 1][:, c0 : c0 + cw],
                start=False,
                stop=True,
                skip_group_check=True,
            )
        yt = att.tile([P, D], F32, tag="y", name="ytile", bufs=4)
        nc.vector.tensor_add(yt[:, 0:512], psA, brep[:, 0:512])
        nc.vector.tensor_add(yt[:, 512:D], psB, brep[:, 512:D])
        # y stores spread over all three DMA queues: a single queue at
        # ~160GB/s would pace the last ~19us of the tail with the 3MB output
        if i < NT - 2:
            nc.scalar.dma_start(out=y[i * P : (i + 1) * P, 0:512], in_=yt[:, 0:512])
            nc.sync.dma_start(out=y[i * P : (i + 1) * P, 512:D], in_=yt[:, 512:D])
        else:
            nc.scalar.dma_start(out=y[i * P : (i + 1) * P, 0:256], in_=yt[:, 0:256])
            nc.sync.dma_start(out=y[i * P : (i + 1) * P, 256:512], in_=yt[:, 256:512])
            nc.gpsimd.dma_start(out=y[i * P : (i + 1) * P, 512:D], in_=yt[:, 512:D])

    kinds = {0: "m", 1: "m", 2: "m", 3: "o"}
    heads = {i: proj_head(i, kinds[i]) for i in range(4)}
    for i in range(NT):
        proj_tail(i, heads.pop(i))
        if i + 4 < NT:
            heads[i + 4] = proj_head(i + 4, kinds[i])


def build_nc(debug: bool = False):
    nc = bacc.Bacc("TRN2", target_bir_lowering=False, debug=debug, enable_asserts=False)
    x = nc.dram_tensor("x", [N, D], F32, kind="ExternalInput").ap()
    w_qkv = nc.dram_tensor("w_qkv", [D, 3 * D], F32, kind="ExternalInput").ap()
    w_proj = nc.dram_tensor("w_proj", [D, D], F32, kind="ExternalInput").ap()
    b_proj = nc.dram_tensor("b_proj", [D], F32, kind="ExternalInput").ap()
    y = nc.dram_tensor("y", [N, D], F32, kind="ExternalOutput").ap()
    with tile.TileContext(nc) as tc:
        with ExitStack() as ctx:
            build_attention(ctx, tc, x, w_qkv, w_proj, b_proj, y)
    nc.compile()
    return nc


_NC = None


def _get_nc():
    global _NC
    if _NC is None:
        _NC = build_nc()
    return _NC


def kernel(inputs, w_qkv, w_proj, b_proj, _trace=False, **run_kwargs):
    from concourse.bass_utils import run_bass_kernel_spmd

    nc = _get_nc()
    inputs = np.asarray(inputs, dtype=np.float32)
    w_qkv = np.ascontiguousarray(np.asarray(w_qkv, dtype=np.float32))
    w_proj = np.ascontiguousarray(np.asarray(w_proj, dtype=np.float32))
    b_proj = np.ascontiguousarray(np.asarray(b_proj, dtype=np.float32))
    in_maps = [
        {
            "x": np.ascontiguousarray(inputs[i]),
            "w_qkv": w_qkv,
            "w_proj": w_proj,
            "b_proj": b_proj,
        }
        for i in range(NCORES)
    ]
    res = run_bass_kernel_spmd(nc, in_maps, list(range(NCORES)), trace=_trace, **run_kwargs)
    out = np.stack([res.results[i]["y"] for i in range(NCORES)], axis=0)
    if _trace:
        return out, res
    return out
